# revision 7
# baseline (speedup 1.0000x reference)
"""Trainium2 Bass kernel for nn_BRCLoss (supervised-contrastive style loss).

Math (per batch sample b, matching the jax reference):
    f = features[b].reshape(24, 4096); fhat = f / ||f||_row
    logits = (fhat @ fhat.T) / 0.1; exp_logits = exp(logits) * (1 - I)
    log_prob = logits - log(exp_logits.sum(-1))
    mlpp = (mask * log_prob).sum(-1) / (mask.sum(-1) + 1e-6)
    loss = sum_b mean_m(-0.1 * mlpp) / 512

Data parallel: 64 samples per core, host adds the 8 partial sums.
Per-core: 13 tiles of [120 rows, 4096] (tile 12 re-reads 24 rows,
zero-weighted).  SWDGE loads cast f32->bf16 in flight; PE transposes +
accumulating bf16 matmuls build per-tile Grams; masked-softmax epilogue per
tile.  The kernel is HBM-stream-bound (~69us of DMA at ~371 GB/s); the
optimization over the plain version is TAIL LATENCY: the last two tiles
compute two small Grams each ([72]+[48] row blocks, 24-row sample-aligned)
instead of one [120] Gram, so the serial epilogue chains that must run
after the final DMA bytes land operate on [48,48]/[72,72] tensors and the
previous tile's chains finish before the stream ends.  Output columns in
t1cols/ldcols: 11 full tiles + 4 sub-blocks = 15; host weights w1/w2 fold
self-exclusion counts, anchor mean, 1/B and duplicate-row zeroing.
"""

import os
import sys

import numpy as np

if "/opt/trn_rl_repo" not in sys.path:
    sys.path.insert(0, "/opt/trn_rl_repo")

B = 512
NV = 2
NCLS = 12
D = 4096
M = NV * NCLS
NCORES = 8
SPC = B // NCORES
ROWS = SPC * M
P = 120
G5 = P // M
T = 13
CH = 128
NCH = D // CH
QUAD = 8
NQ = NCH // QUAD
TEMP = 0.1
EPS_POS = 1e-6

_ROW_STARTS = [P * t for t in range(T - 1)] + [ROWS - P]
# Per-output-column blocks: full tiles 0..10, then [72]+[48] splits of
# tiles 11 and 12 so the post-stream epilogue chains are small.
BLOCKS = [(t, 0, P) for t in range(T - 2)] + [
    (T - 2, 0, 72), (T - 2, 72, P), (T - 1, 0, 72), (T - 1, 72, P)]
NCOLS = len(BLOCKS)

_compiled = None
LAST_RESULTS = None


def _host_consts():
    i = np.arange(NCLS)
    graph = (np.abs(i[:, None] - i[None, :]) <= 1).astype(np.float32)
    eye24 = np.eye(M, dtype=np.float32)
    mask24 = np.tile(graph, (NV, NV)) * (1.0 - eye24)
    blk = np.kron(np.eye(G5, dtype=np.float32), np.ones((M, M), np.float32))
    m0 = (blk * (1.0 - np.eye(P, dtype=np.float32))).astype(np.float32)
    pm = np.kron(np.eye(G5, dtype=np.float32), mask24).astype(np.float32)
    im = (TEMP * np.eye(P)).astype(np.float32)
    msum = np.tile(mask24.sum(1), G5).astype(np.float64)
    alpha = -TEMP / ((msum + EPS_POS) * M * B)
    w1 = np.zeros((P, NCOLS), np.float32)
    w2 = np.zeros((P, NCOLS), np.float32)
    for c, (t, lo, hi) in enumerate(BLOCKS):
        p = hi - lo
        w1[:p, c] = alpha[lo:hi]
        w2[:p, c] = -(alpha * msum)[lo:hi]
        if t == T - 1 and lo == 0:
            w1[:M, c] = 0.0          # duplicated rows 1416-1439
            w2[:M, c] = 0.0
    return {"m0": m0, "pm": pm, "im": im, "w1": w1, "w2": w2}


def _build():
    from contextlib import ExitStack

    from concourse import bacc, bass, masks, mybir, tile

    f32 = mybir.dt.float32
    bf16 = mybir.dt.bfloat16
    AX = mybir.AxisListType
    ALU = mybir.AluOpType
    ACTF = mybir.ActivationFunctionType

    import bass_rust as _bass_rust
    from concourse.hw_specs import get_activation_tables

    class _OneActSetBacc(bacc.Bacc):
        def insert_act_table_loads(self):
            has_activation = any(
                isinstance(i, mybir.InstActivation)
                for b in self.main_func.blocks
                for i in b.instructions
            )
            if not has_activation:
                return
            tables = [
                (n, (s if n == "natural_log_exp_and_others" else set()))
                for n, s in get_activation_tables(self.m.arch).items()
            ]
            _bass_rust.insert_act_table_loads(self, tables)

    nc = _OneActSetBacc("TRN2", target_bir_lowering=False, debug=False,
                        num_devices=NCORES)

    f_dram = nc.dram_tensor("f", (ROWS, D), f32, kind="ExternalInput")
    m0_dram = nc.dram_tensor("m0", (P, P), f32, kind="ExternalInput")
    pm_dram = nc.dram_tensor("pm", (P, P), f32, kind="ExternalInput")
    im_dram = nc.dram_tensor("im", (P, P), f32, kind="ExternalInput")
    w1_dram = nc.dram_tensor("w1", (P, NCOLS), f32, kind="ExternalInput")
    w2_dram = nc.dram_tensor("w2", (P, NCOLS), f32, kind="ExternalInput")
    out_dram = nc.dram_tensor("out", (1, 1), f32, kind="ExternalOutput")

    DSPLIT = 2
    DCOLS = D // DSPLIT

    with ExitStack() as ctx:
        tc = ctx.enter_context(tile.TileContext(nc))
        consts = ctx.enter_context(tc.tile_pool(name="consts", bufs=1))
        fpool = ctx.enter_context(tc.tile_pool(name="fpool", bufs=8))
        tcpool = ctx.enter_context(tc.tile_pool(name="tcpool", bufs=5))
        work = ctx.enter_context(tc.tile_pool(name="work", bufs=1))
        lwork = ctx.enter_context(tc.tile_pool(name="lwork", bufs=2))
        small = ctx.enter_context(tc.tile_pool(name="small", bufs=2))
        tpsum = ctx.enter_context(
            tc.tile_pool(name="tpsum", bufs=4, space=bass.MemorySpace.PSUM))
        gpsum = ctx.enter_context(
            tc.tile_pool(name="gpsum", bufs=2, space=bass.MemorySpace.PSUM))
        rpsum = ctx.enter_context(
            tc.tile_pool(name="rpsum", bufs=2, space=bass.MemorySpace.PSUM))

        def load_tile(ft, t):
            r0 = _ROW_STARTS[t]
            nsp = 8 if t == T - 1 else (4 if t == T - 2 else DSPLIT)
            w = D // nsp
            for q in range(nsp):
                nc.gpsimd.dma_start(ft[:, q * w:(q + 1) * w],
                                    f_dram[r0:r0 + P, q * w:(q + 1) * w])

        ftiles = []
        for t in range(T):
            ft = fpool.tile([P, D], bf16, tag="f")
            if t < 3:
                load_tile(ft, t)
            ftiles.append(ft)

        identb = consts.tile([128, 128], bf16, tag="identb")
        masks.make_identity(nc, identb[:])
        m0_t = consts.tile([P, P], f32, tag="m0")
        pm_t = consts.tile([P, P], f32, tag="pm")
        im_t = consts.tile([P, P], f32, tag="im")
        w1_t = consts.tile([P, NCOLS], f32, tag="w1")
        w2_t = consts.tile([P, NCOLS], f32, tag="w2")
        nc.scalar.dma_start(m0_t[:], m0_dram[:, :])
        nc.scalar.dma_start(pm_t[:], pm_dram[:, :])
        nc.scalar.dma_start(im_t[:], im_dram[:, :])
        nc.scalar.dma_start(w1_t[:], w1_dram[:, :])
        nc.scalar.dma_start(w2_t[:], w2_dram[:, :])

        warm = consts.tile([1, 2], f32, tag="warm")
        nc.vector.memset(warm[:], 1.0)
        nc.scalar.activation(warm[:, 1:2], warm[:, 0:1], ACTF.Exp)

        t1cols = work.tile([P, NCOLS], f32, tag="t1cols")
        ldcols = work.tile([P, NCOLS], f32, tag="ldcols")
        # Sub-block columns only cover hi-lo rows; zero everything so the
        # final weighted reduce never sees stale SBUF under 0-weights.
        nc.vector.memset(t1cols[:], 0.0)
        nc.vector.memset(ldcols[:], 0.0)
        egpool = ctx.enter_context(tc.tile_pool(name="egpool", bufs=4))
        egs = {}

        def tile_gram(t, blocks):
            ft = ftiles[t]
            if t >= 3:
                load_tile(ft, t)
            gs = [gpsum.tile([hi - lo, hi - lo], f32, tag="g", name=f"g{t}_{lo}")
                  for (_, lo, hi) in blocks]
            tcs_list = []
            interleave = (t == T - 1)

            def mm_chunk(c, last):
                tcs = tcs_list[c // QUAD]
                for g, (_, lo, hi) in zip(gs, blocks):
                    sl = tcs[:, (c % QUAD) * P + lo:(c % QUAD) * P + hi]
                    nc.tensor.matmul(g[:], sl, sl, start=(c == 0), stop=last)

            for q in range(NQ):
                tp = tpsum.tile([128, QUAD * P], bf16, tag="tp")
                for j in range(QUAD):
                    c = q * QUAD + j
                    nc.tensor.transpose(
                        tp[:, j * P:(j + 1) * P],
                        ft[:, c * CH:(c + 1) * CH],
                        identb[:P, :P],
                    )
                tcs = tcpool.tile([128, QUAD * P], bf16, tag="tc")
                if q % 2 == 0:
                    nc.vector.tensor_copy(tcs[:], tp[:])
                else:
                    nc.scalar.copy(tcs[:], tp[:])
                tcs_list.append(tcs)
                if interleave:
                    for j in range(QUAD):
                        c = q * QUAD + j
                        mm_chunk(c, c == NCH - 1)
            if not interleave:
                for c in range(NCH):
                    mm_chunk(c, c == NCH - 1)
            outs = []
            for g, (_, lo, hi) in zip(gs, blocks):
                p = hi - lo
                eg = egpool.tile([p, p], bf16, tag="eg")
                nc.vector.tensor_copy(eg[:], g[:])
                scr = lwork.tile([p, p], f32, tag="scr")
                nc.vector.tensor_tensor(scr[:], g[:], im_t[:p, :p],
                                        ALU.mult)
                d2 = small.tile([p, 1], f32, tag="d2")
                nc.vector.tensor_reduce(d2[:], scr[:], axis=AX.X, op=ALU.add)
                outs.append((eg, d2))
            return outs

        def block_softmax(col, lo, hi, eg, d2):
            p = hi - lo
            lnv = small.tile([p, 1], f32, tag="lnv")
            nc.scalar.activation(lnv[:], d2[:], ACTF.Ln)
            rnx = small.tile([p, 1], f32, tag="rnx")
            nc.scalar.activation(rnx[:], lnv[:], ACTF.Exp, scale=-0.5)
            drn = lwork.tile([p, p], bf16, tag="drn")
            nc.vector.tensor_scalar(drn[:], im_t[:p, :p], rnx[:],
                                    1.0 / TEMP, op0=ALU.mult, op1=ALU.mult)
            h_ps = rpsum.tile([p, p], f32, tag="r")
            nc.tensor.matmul(h_ps[:], eg[:], drn[:], start=True, stop=True)
            lt = lwork.tile([p, p], f32, tag="lt")
            nc.vector.tensor_scalar_mul(lt[:], h_ps[:], rnx[:])
            xt = lwork.tile([p, p], f32, tag="xt")
            nc.scalar.activation(xt[:], lt[:], ACTF.Exp)
            xm = lwork.tile([p, p], f32, tag="xm")
            nc.vector.tensor_tensor(xm[:], xt[:], m0_t[:p, :p], ALU.mult)
            st = small.tile([p, 1], f32, tag="st")
            nc.vector.tensor_reduce(st[:], xm[:], axis=AX.X, op=ALU.add)
            nc.scalar.activation(ldcols[:p, col:col + 1], st[:], ACTF.Ln)
            lp = lwork.tile([p, p], f32, tag="lp")
            nc.vector.tensor_tensor(lp[:], lt[:], pm_t[:p, :p], ALU.mult)
            nc.vector.tensor_reduce(t1cols[:p, col:col + 1], lp[:], axis=AX.X,
                                    op=ALU.add)

        col = 0
        for t in range(T):
            blocks = [b for b in BLOCKS if b[0] == t]
            outs = tile_gram(t, blocks)
            for (bt, lo, hi), (eg, d2) in zip(blocks, outs):
                block_softmax(col, lo, hi, eg, d2)
                col += 1
        assert col == NCOLS

        ld = ldcols
        z1 = work.tile([P, NCOLS], f32, tag="z1")
        nc.vector.tensor_tensor(z1[:], t1cols[:], w1_t[:], ALU.mult)
        z2 = work.tile([P, NCOLS], f32, tag="z2")
        nc.vector.tensor_tensor(z2[:], ld[:], w2_t[:], ALU.mult)
        zs = work.tile([P, NCOLS], f32, tag="zs")
        nc.vector.tensor_add(zs[:], z1[:], z2[:])
        zc = work.tile([P, 1], f32, tag="zc")
        nc.vector.tensor_reduce(zc[:], zs[:], axis=AX.X, op=ALU.add)

        ones = work.tile([P, 1], f32, tag="ones")
        nc.vector.memset(ones[:], 1.0)
        tot_ps = gpsum.tile([1, 1], f32, tag="g")
        nc.tensor.matmul(tot_ps[:, :], zc[:], ones[:], start=True, stop=True)
        tot = work.tile([1, 1], f32, tag="tot")
        nc.vector.tensor_copy(tot[:], tot_ps[:, :])
        nc.sync.dma_start(out_dram[:, :], tot[:])

    nc.compile()
    return nc


def _ensure_axon_hooks():
    try:
        import antenv.axon_hooks  # noqa: F401
        return
    except ImportError:
        pass
    import contextlib
    import ctypes
    import types

    import antenv

    hook = None
    so_path = "/opt/axon/libaxon_pjrt.so"
    try:
        lib = ctypes.CDLL(so_path)
        if hasattr(lib, "axon_start_nrt_profile"):
            lib.axon_start_nrt_profile.argtypes = [
                ctypes.POINTER(ctypes.c_int64), ctypes.c_size_t]
            lib.axon_start_nrt_profile.restype = ctypes.c_int64
            lib.axon_stop_nrt_profile.argtypes = [ctypes.c_char_p]
            lib.axon_stop_nrt_profile.restype = ctypes.c_int64

            @contextlib.contextmanager
            def _hook(output_dir, device_ids):
                import jax
                jax.devices()
                if device_ids:
                    ids = (ctypes.c_int64 * len(device_ids))(*device_ids)
                    rc = lib.axon_start_nrt_profile(ids, len(device_ids))
                else:
                    rc = lib.axon_start_nrt_profile(None, 0)
                if rc != 0:
                    raise RuntimeError(f"axon_start_nrt_profile rc={rc}")
                try:
                    yield
                finally:
                    n = lib.axon_stop_nrt_profile(str(output_dir).encode())
                    print(f"profile: {n} file(s) written to {output_dir}",
                          file=sys.stderr)

            hook = _hook
    except OSError:
        pass

    mod = types.ModuleType("antenv.axon_hooks")
    state = {"hook": hook}
    mod.get_axon_ntff_profile_hook = lambda: state["hook"]
    mod.set_axon_ntff_profile_hook = lambda h: state.__setitem__("hook", h)
    sys.modules["antenv.axon_hooks"] = mod
    antenv.axon_hooks = mod


def kernel(**inputs):
    global _compiled, LAST_RESULTS
    from concourse import bass_utils

    feats = np.ascontiguousarray(
        np.asarray(inputs["features"], dtype=np.float32).reshape(B * M, D))

    if _compiled is None:
        _compiled = (_build(), _host_consts())
    nc, consts = _compiled

    in_maps = []
    for k in range(NCORES):
        im = dict(consts)
        im["f"] = feats[k * ROWS:(k + 1) * ROWS]
        in_maps.append(im)

    trace = bool(os.environ.get("BASS_TRACE"))
    if trace:
        _ensure_axon_hooks()
    try:
        res = bass_utils.run_bass_kernel_spmd(
            nc, in_maps, core_ids=list(range(NCORES)), trace=trace)
    except Exception:
        os.environ["BASS_NEVER_TRACE"] = "1"
        try:
            res = bass_utils.run_bass_kernel_spmd(
                nc, in_maps, core_ids=list(range(NCORES)), trace=False)
        finally:
            del os.environ["BASS_NEVER_TRACE"]
    LAST_RESULTS = res
    total = float(np.sum([np.float64(r["out"][0, 0]) for r in res.results]))
    return np.array(total, dtype=np.float32)


# revision 9
# speedup vs baseline: 1.2030x; 1.2030x over previous
"""Trainium2 Bass kernel for nn_BRCLoss (supervised-contrastive style loss).

Math (per batch sample b, matching the jax reference):
    f = features[b].reshape(24, 4096); fhat = f / ||f||_row
    logits = (fhat @ fhat.T) / 0.1; exp_logits = exp(logits) * (1 - I)
    log_prob = logits - log(exp_logits.sum(-1))
    mlpp = (mask * log_prob).sum(-1) / (mask.sum(-1) + 1e-6)
    loss = sum_b mean_m(-0.1 * mlpp) / 512

Data parallel: 64 samples per core; the host adds the 8 partial sums.
Per-core: 13 tiles of [120 rows, 4096] (tile 12 re-reads 24 rows,
zero-weighted).  SWDGE loads cast f32->bf16 in flight (the HBM stream is
the roofline: ~69us at ~371 GB/s); PE transposes + accumulating bf16
matmuls build per-tile Grams G; a masked-softmax epilogue per tile feeds
one final weighted dot product.

Tail-latency optimizations over the plain version (the only part of the
run not hidden under the DMA stream):
  - The epilogue consumes H = G@diag(rnx) straight from PSUM: exp runs as
    ACT Exp(H, scale=rnx) (per-partition scale carries the row normalizer)
    and the positives sum multiplies H by the mask first, applying rnx
    after the row-reduce on a [*,1] vector - the full-size logits
    intermediate of the reference formulation is never materialized.
  - The last two tiles' epilogue chains are split into partition halves
    (bases 0 and 64), so the serial op chain after the final DMA bytes
    land runs on half-height tensors with the two halves pipelined
    across DVE/ACT.
  - The final weighted reduce over tiles 0..11 is issued before the last
    tile's chain; only the last column's [120,1] contribution remains on
    the critical path.
"""

import os
import sys

import numpy as np

if "/opt/trn_rl_repo" not in sys.path:
    sys.path.insert(0, "/opt/trn_rl_repo")

B = 512
NV = 2
NCLS = 12
D = 4096
M = NV * NCLS
NCORES = 8
SPC = B // NCORES
ROWS = SPC * M
P = 120
G5 = P // M
T = 13
CH = 128
NCH = D // CH
QUAD = 8
NQ = NCH // QUAD
TEMP = 0.1
EPS_POS = 1e-6

_ROW_STARTS = [P * t for t in range(T - 1)] + [ROWS - P]

_compiled = None
LAST_RESULTS = None


def _host_consts():
    i = np.arange(NCLS)
    graph = (np.abs(i[:, None] - i[None, :]) <= 1).astype(np.float32)
    eye24 = np.eye(M, dtype=np.float32)
    mask24 = np.tile(graph, (NV, NV)) * (1.0 - eye24)
    blk = np.kron(np.eye(G5, dtype=np.float32), np.ones((M, M), np.float32))
    m0 = (blk * (1.0 - np.eye(P, dtype=np.float32))).astype(np.float32)
    pm = np.kron(np.eye(G5, dtype=np.float32), mask24).astype(np.float32)
    im = (TEMP * np.eye(P)).astype(np.float32)
    msum = np.tile(mask24.sum(1), G5).astype(np.float64)
    alpha = -TEMP / ((msum + EPS_POS) * M * B)
    valid = np.ones((P, T), np.float64)
    valid[:M, T - 1] = 0.0
    w1 = (alpha[:, None] * valid).astype(np.float32)
    w2 = ((-alpha * msum)[:, None] * valid).astype(np.float32)
    return {"m0": m0, "pm": pm, "im": im, "w1": w1, "w2": w2}


def _build():
    from contextlib import ExitStack

    from concourse import bacc, bass, masks, mybir, tile

    f32 = mybir.dt.float32
    bf16 = mybir.dt.bfloat16
    AX = mybir.AxisListType
    ALU = mybir.AluOpType
    ACTF = mybir.ActivationFunctionType

    import bass_rust as _bass_rust
    from concourse.hw_specs import get_activation_tables

    class _OneActSetBacc(bacc.Bacc):
        def insert_act_table_loads(self):
            has_activation = any(
                isinstance(i, mybir.InstActivation)
                for b in self.main_func.blocks
                for i in b.instructions
            )
            if not has_activation:
                return
            tables = [
                (n, (s if n == "natural_log_exp_and_others" else set()))
                for n, s in get_activation_tables(self.m.arch).items()
            ]
            _bass_rust.insert_act_table_loads(self, tables)

    nc = _OneActSetBacc("TRN2", target_bir_lowering=False, debug=False,
                        num_devices=NCORES)

    f_dram = nc.dram_tensor("f", (ROWS, D), f32, kind="ExternalInput")
    m0_dram = nc.dram_tensor("m0", (P, P), f32, kind="ExternalInput")
    pm_dram = nc.dram_tensor("pm", (P, P), f32, kind="ExternalInput")
    im_dram = nc.dram_tensor("im", (P, P), f32, kind="ExternalInput")
    w1_dram = nc.dram_tensor("w1", (P, T), f32, kind="ExternalInput")
    w2_dram = nc.dram_tensor("w2", (P, T), f32, kind="ExternalInput")
    out_dram = nc.dram_tensor("out", (1, 1), f32, kind="ExternalOutput")

    DSPLIT = 2
    DCOLS = D // DSPLIT

    with ExitStack() as ctx:
        tc = ctx.enter_context(tile.TileContext(nc))
        consts = ctx.enter_context(tc.tile_pool(name="consts", bufs=1))
        fpool = ctx.enter_context(tc.tile_pool(name="fpool", bufs=8))
        tcpool = ctx.enter_context(tc.tile_pool(name="tcpool", bufs=5))
        work = ctx.enter_context(tc.tile_pool(name="work", bufs=1))
        lwork = ctx.enter_context(tc.tile_pool(name="lwork", bufs=2))
        small = ctx.enter_context(tc.tile_pool(name="small", bufs=2))
        tpsum = ctx.enter_context(
            tc.tile_pool(name="tpsum", bufs=4, space=bass.MemorySpace.PSUM))
        gpsum = ctx.enter_context(
            tc.tile_pool(name="gpsum", bufs=2, space=bass.MemorySpace.PSUM))
        rpsum = ctx.enter_context(
            tc.tile_pool(name="rpsum", bufs=2, space=bass.MemorySpace.PSUM))

        def load_tile(ft, t):
            r0 = _ROW_STARTS[t]
            nsp = 8 if t == T - 1 else (4 if t == T - 2 else DSPLIT)
            w = D // nsp
            for q in range(nsp):
                nc.gpsimd.dma_start(ft[:, q * w:(q + 1) * w],
                                    f_dram[r0:r0 + P, q * w:(q + 1) * w])

        ftiles = []
        for t in range(T):
            ft = fpool.tile([P, D], bf16, tag="f")
            if t < 3:
                load_tile(ft, t)
            ftiles.append(ft)

        identb = consts.tile([128, 128], bf16, tag="identb")
        masks.make_identity(nc, identb[:])
        m0_t = consts.tile([P, P], f32, tag="m0")
        pm_t = consts.tile([P, P], f32, tag="pm")
        im_t = consts.tile([P, P], f32, tag="im")
        w1_t = consts.tile([P, T], f32, tag="w1")
        w2_t = consts.tile([P, T], f32, tag="w2")
        nc.scalar.dma_start(m0_t[:], m0_dram[:, :])
        nc.scalar.dma_start(pm_t[:], pm_dram[:, :])
        nc.scalar.dma_start(im_t[:], im_dram[:, :])
        nc.scalar.dma_start(w1_t[:], w1_dram[:, :])
        nc.scalar.dma_start(w2_t[:], w2_dram[:, :])

        warm = consts.tile([1, 2], f32, tag="warm")
        nc.vector.memset(warm[:], 1.0)
        nc.scalar.activation(warm[:, 1:2], warm[:, 0:1], ACTF.Exp)

        t1cols = work.tile([P, T], f32, tag="t1cols")
        ldcols = work.tile([P, T], f32, tag="ldcols")
        egpool = ctx.enter_context(tc.tile_pool(name="egpool", bufs=4))
        egs = {}

        def tile_gram(t):
            ft = ftiles[t]
            if t >= 3:
                load_tile(ft, t)
            g = gpsum.tile([P, P], f32, tag="g")
            tcs_list = []
            interleave = (t == T - 1)
            for q in range(NQ):
                tp = tpsum.tile([128, QUAD * P], bf16, tag="tp")
                for j in range(QUAD):
                    c = q * QUAD + j
                    nc.tensor.transpose(
                        tp[:, j * P:(j + 1) * P],
                        ft[:, c * CH:(c + 1) * CH],
                        identb[:P, :P],
                    )
                tcs = tcpool.tile([128, QUAD * P], bf16, tag="tc")
                if q % 2 == 0:
                    nc.vector.tensor_copy(tcs[:], tp[:])
                else:
                    nc.scalar.copy(tcs[:], tp[:])
                tcs_list.append(tcs)
                if interleave:
                    for j in range(QUAD):
                        c = q * QUAD + j
                        sl = tcs[:, j * P:(j + 1) * P]
                        nc.tensor.matmul(g[:], sl, sl,
                                         start=(c == 0), stop=(c == NCH - 1))
            if not interleave:
                for c in range(NCH):
                    sl = tcs_list[c // QUAD][:, (c % QUAD) * P:(c % QUAD + 1) * P]
                    nc.tensor.matmul(g[:], sl, sl,
                                     start=(c == 0), stop=(c == NCH - 1))
            eg = egpool.tile([P, P], bf16, tag="eg")
            nc.vector.tensor_copy(eg[:], g[:])
            egs[t] = eg
            scr = lwork.tile([P, P], f32, tag="scr")
            nc.vector.tensor_tensor(scr[:], g[:], im_t[:], ALU.mult)
            d2 = small.tile([P, 1], f32, tag="d2")
            nc.vector.tensor_reduce(d2[:], scr[:], axis=AX.X, op=ALU.add)
            return d2

        def tile_softmax(t, d2):
            # rnx = (0.1*diag G)^-0.5; H = G @ diag(rnx) on the PE (G is
            # symmetric so lhsT=G); the row scale rides the ACT Exp for the
            # denominator and a post-reduce [*,1] multiply for the
            # positives, so no full-size logits tensor is built.  For the
            # last two tiles every H-consumer is issued as two
            # partition-half ops (bases 0/64) so the post-stream chain is
            # half-height and the halves pipeline across DVE/ACT.
            eg = egs.pop(t)
            lnv = small.tile([P, 1], f32, tag="lnv")
            nc.scalar.activation(lnv[:], d2[:], ACTF.Ln)
            rnx = small.tile([P, 1], f32, tag="rnx")
            nc.scalar.activation(rnx[:], lnv[:], ACTF.Exp, scale=-0.5)
            drn = lwork.tile([P, P], bf16, tag="drn")
            nc.vector.tensor_scalar(drn[:], im_t[:], rnx[:], 1.0 / TEMP,
                                    op0=ALU.mult, op1=ALU.mult)
            h_ps = rpsum.tile([P, P], f32, tag="r")
            nc.tensor.matmul(h_ps[:], eg[:], drn[:], start=True, stop=True)
            halves = [(0, 64), (64, P)] if t >= T - 2 else [(0, P)]
            xt = lwork.tile([P, P], f32, tag="xt")
            xm = lwork.tile([P, P], f32, tag="xm")
            lp = lwork.tile([P, P], f32, tag="lp")
            st = small.tile([P, 1], f32, tag="st")
            lr = small.tile([P, 1], f32, tag="lr")
            for a, b in halves:
                nc.scalar.activation(xt[a:b, :], h_ps[a:b, :], ACTF.Exp,
                                     scale=rnx[a:b])
            for a, b in halves:
                nc.vector.tensor_tensor(xm[a:b, :], xt[a:b, :], m0_t[a:b, :],
                                        ALU.mult)
            for a, b in halves:
                nc.vector.tensor_reduce(st[a:b], xm[a:b, :], axis=AX.X,
                                        op=ALU.add)
            for a, b in halves:
                nc.scalar.activation(ldcols[a:b, t:t + 1], st[a:b], ACTF.Ln)
            for a, b in halves:
                nc.vector.tensor_tensor(lp[a:b, :], h_ps[a:b, :], pm_t[a:b, :],
                                        ALU.mult)
            for a, b in halves:
                nc.vector.tensor_reduce(lr[a:b], lp[a:b, :], axis=AX.X,
                                        op=ALU.add)
            for a, b in halves:
                nc.vector.tensor_scalar_mul(t1cols[a:b, t:t + 1], lr[a:b],
                                            rnx[a:b])

        L = T - 1
        z1 = work.tile([P, T], f32, tag="z1")
        z2 = work.tile([P, T], f32, tag="z2")
        zs = work.tile([P, T], f32, tag="zs")
        zca = work.tile([P, 1], f32, tag="zca")
        for t in range(T):
            d2 = tile_gram(t)
            tile_softmax(t, d2)
            if t == T - 2:
                # Emit the bulk weighted reduce (cols 0..T-2) before the
                # last tile's chain so it runs under the stream tail -
                # engine queues are FIFO, late emission would serialize it
                # behind the final chain.
                nc.vector.tensor_tensor(z1[:, :L], t1cols[:, :L], w1_t[:, :L],
                                        ALU.mult)
                nc.vector.tensor_tensor(z2[:, :L], ldcols[:, :L], w2_t[:, :L],
                                        ALU.mult)
                nc.vector.tensor_add(zs[:, :L], z1[:, :L], z2[:, :L])
                nc.vector.tensor_reduce(zca[:], zs[:, :L], axis=AX.X,
                                        op=ALU.add)

        # Only the last tile's [120,1] contribution stays on the tail path.
        nc.vector.tensor_tensor(z1[:, L:], t1cols[:, L:], w1_t[:, L:], ALU.mult)
        nc.vector.tensor_tensor(z2[:, L:], ldcols[:, L:], w2_t[:, L:], ALU.mult)
        zsl = work.tile([P, 1], f32, tag="zsl")
        nc.vector.tensor_add(zsl[:], z1[:, L:], z2[:, L:])
        zc = work.tile([P, 1], f32, tag="zc")
        nc.vector.tensor_add(zc[:], zca[:], zsl[:])

        ones = work.tile([P, 1], f32, tag="ones")
        nc.vector.memset(ones[:], 1.0)
        tot_ps = gpsum.tile([1, 1], f32, tag="g")
        nc.tensor.matmul(tot_ps[:, :], zc[:], ones[:], start=True, stop=True)
        tot = work.tile([1, 1], f32, tag="tot")
        nc.vector.tensor_copy(tot[:], tot_ps[:, :])
        nc.sync.dma_start(out_dram[:, :], tot[:])

    nc.compile()
    return nc


def _ensure_axon_hooks():
    try:
        import antenv.axon_hooks  # noqa: F401
        return
    except ImportError:
        pass
    import contextlib
    import ctypes
    import types

    import antenv

    hook = None
    so_path = "/opt/axon/libaxon_pjrt.so"
    try:
        lib = ctypes.CDLL(so_path)
        if hasattr(lib, "axon_start_nrt_profile"):
            lib.axon_start_nrt_profile.argtypes = [
                ctypes.POINTER(ctypes.c_int64), ctypes.c_size_t]
            lib.axon_start_nrt_profile.restype = ctypes.c_int64
            lib.axon_stop_nrt_profile.argtypes = [ctypes.c_char_p]
            lib.axon_stop_nrt_profile.restype = ctypes.c_int64

            @contextlib.contextmanager
            def _hook(output_dir, device_ids):
                import jax
                jax.devices()
                if device_ids:
                    ids = (ctypes.c_int64 * len(device_ids))(*device_ids)
                    rc = lib.axon_start_nrt_profile(ids, len(device_ids))
                else:
                    rc = lib.axon_start_nrt_profile(None, 0)
                if rc != 0:
                    raise RuntimeError(f"axon_start_nrt_profile rc={rc}")
                try:
                    yield
                finally:
                    n = lib.axon_stop_nrt_profile(str(output_dir).encode())
                    print(f"profile: {n} file(s) written to {output_dir}",
                          file=sys.stderr)

            hook = _hook
    except OSError:
        pass

    mod = types.ModuleType("antenv.axon_hooks")
    state = {"hook": hook}
    mod.get_axon_ntff_profile_hook = lambda: state["hook"]
    mod.set_axon_ntff_profile_hook = lambda h: state.__setitem__("hook", h)
    sys.modules["antenv.axon_hooks"] = mod
    antenv.axon_hooks = mod


def kernel(**inputs):
    global _compiled, LAST_RESULTS
    from concourse import bass_utils

    feats = np.ascontiguousarray(
        np.asarray(inputs["features"], dtype=np.float32).reshape(B * M, D))

    if _compiled is None:
        _compiled = (_build(), _host_consts())
    nc, consts = _compiled

    in_maps = []
    for k in range(NCORES):
        im = dict(consts)
        im["f"] = feats[k * ROWS:(k + 1) * ROWS]
        in_maps.append(im)

    trace = bool(os.environ.get("BASS_TRACE"))
    if trace:
        _ensure_axon_hooks()
    try:
        res = bass_utils.run_bass_kernel_spmd(
            nc, in_maps, core_ids=list(range(NCORES)), trace=trace)
    except Exception:
        os.environ["BASS_NEVER_TRACE"] = "1"
        try:
            res = bass_utils.run_bass_kernel_spmd(
                nc, in_maps, core_ids=list(range(NCORES)), trace=False)
        finally:
            del os.environ["BASS_NEVER_TRACE"]
    LAST_RESULTS = res
    total = float(np.sum([np.float64(r["out"][0, 0]) for r in res.results]))
    return np.array(total, dtype=np.float32)


# revision 11
# speedup vs baseline: 1.2572x; 1.0451x over previous
"""Trainium2 Bass kernel for nn_BRCLoss (supervised-contrastive style loss).

Math (per batch sample b, matching the jax reference):
    f = features[b].reshape(24, 4096); fhat = f / ||f||_row
    logits = (fhat @ fhat.T) / 0.1; exp_logits = exp(logits) * (1 - I)
    log_prob = logits - log(exp_logits.sum(-1))
    mlpp = (mask * log_prob).sum(-1) / (mask.sum(-1) + 1e-6)
    loss = sum_b mean_m(-0.1 * mlpp) / 512

Data parallel: 64 samples per core; the host adds the 8 partial sums.
Per-core: 13 tiles of [120 rows, 4096] (tile 12 re-reads 24 rows,
zero-weighted).  SWDGE loads cast f32->bf16 in flight (the HBM stream is
the roofline: ~69us at ~371 GB/s); PE transposes + accumulating bf16
matmuls build per-tile Grams G; a masked-softmax epilogue per tile feeds
one final weighted dot product.

Tail-latency optimizations over the plain version (the only part of the
run not hidden under the DMA stream):
  - The epilogue consumes H = G@diag(rnx) straight from PSUM: exp runs as
    ACT Exp(H, scale=rnx) (per-partition scale carries the row normalizer)
    and the positives sum multiplies H by the mask first, applying rnx
    after the row-reduce on a [*,1] vector - the full-size logits
    intermediate of the reference formulation is never materialized.
  - The last two tiles' epilogue chains are split into partition halves
    (bases 0 and 64), so the serial op chain after the final DMA bytes
    land runs on half-height tensors with the two halves pipelined
    across DVE/ACT.
  - The final weighted reduce over tiles 0..11 is issued before the last
    tile's chain; only the last column's [120,1] contribution remains on
    the critical path.
"""

import os
import sys

import numpy as np

if "/opt/trn_rl_repo" not in sys.path:
    sys.path.insert(0, "/opt/trn_rl_repo")

B = 512
NV = 2
NCLS = 12
D = 4096
M = NV * NCLS
NCORES = 8
SPC = B // NCORES
ROWS = SPC * M
P = 120
G5 = P // M
T = 13
CH = 128
NCH = D // CH
QUAD = 8
NQ = NCH // QUAD
TEMP = 0.1
EPS_POS = 1e-6

_ROW_STARTS = [P * t for t in range(T - 1)] + [ROWS - P]

_compiled = None
LAST_RESULTS = None


def _host_consts():
    i = np.arange(NCLS)
    graph = (np.abs(i[:, None] - i[None, :]) <= 1).astype(np.float32)
    eye24 = np.eye(M, dtype=np.float32)
    mask24 = np.tile(graph, (NV, NV)) * (1.0 - eye24)
    blk = np.kron(np.eye(G5, dtype=np.float32), np.ones((M, M), np.float32))
    m0 = (blk * (1.0 - np.eye(P, dtype=np.float32))).astype(np.float32)
    pm = np.kron(np.eye(G5, dtype=np.float32), mask24).astype(np.float32)
    im = (TEMP * np.eye(P)).astype(np.float32)
    msum = np.tile(mask24.sum(1), G5).astype(np.float64)
    alpha = -TEMP / ((msum + EPS_POS) * M * B)
    valid = np.ones((P, T), np.float64)
    valid[:M, T - 1] = 0.0
    w1 = (alpha[:, None] * valid).astype(np.float32)
    w2 = ((-alpha * msum)[:, None] * valid).astype(np.float32)
    return {"m0": m0, "pm": pm, "im": im, "w1": w1, "w2": w2}


def _build():
    from contextlib import ExitStack

    from concourse import bacc, bass, masks, mybir, tile

    f32 = mybir.dt.float32
    bf16 = mybir.dt.bfloat16
    AX = mybir.AxisListType
    ALU = mybir.AluOpType
    ACTF = mybir.ActivationFunctionType

    import bass_rust as _bass_rust
    from concourse.hw_specs import get_activation_tables

    class _OneActSetBacc(bacc.Bacc):
        def insert_act_table_loads(self):
            has_activation = any(
                isinstance(i, mybir.InstActivation)
                for b in self.main_func.blocks
                for i in b.instructions
            )
            if not has_activation:
                return
            tables = [
                (n, (s if n == "natural_log_exp_and_others" else set()))
                for n, s in get_activation_tables(self.m.arch).items()
            ]
            _bass_rust.insert_act_table_loads(self, tables)

    nc = _OneActSetBacc("TRN2", target_bir_lowering=False, debug=False,
                        num_devices=NCORES)

    f_dram = nc.dram_tensor("f", (ROWS, D), f32, kind="ExternalInput")
    m0_dram = nc.dram_tensor("m0", (P, P), f32, kind="ExternalInput")
    pm_dram = nc.dram_tensor("pm", (P, P), f32, kind="ExternalInput")
    im_dram = nc.dram_tensor("im", (P, P), f32, kind="ExternalInput")
    w1_dram = nc.dram_tensor("w1", (P, T), f32, kind="ExternalInput")
    w2_dram = nc.dram_tensor("w2", (P, T), f32, kind="ExternalInput")
    out_dram = nc.dram_tensor("out", (1, 1), f32, kind="ExternalOutput")

    DSPLIT = 2
    DCOLS = D // DSPLIT

    with ExitStack() as ctx:
        tc = ctx.enter_context(tile.TileContext(nc))
        consts = ctx.enter_context(tc.tile_pool(name="consts", bufs=1))
        fpool = ctx.enter_context(tc.tile_pool(name="fpool", bufs=8))
        tcpool = ctx.enter_context(tc.tile_pool(name="tcpool", bufs=5))
        work = ctx.enter_context(tc.tile_pool(name="work", bufs=1))
        lwork = ctx.enter_context(tc.tile_pool(name="lwork", bufs=2))
        small = ctx.enter_context(tc.tile_pool(name="small", bufs=2))
        tpsum = ctx.enter_context(
            tc.tile_pool(name="tpsum", bufs=4, space=bass.MemorySpace.PSUM))
        gpsum = ctx.enter_context(
            tc.tile_pool(name="gpsum", bufs=2, space=bass.MemorySpace.PSUM))
        rpsum = ctx.enter_context(
            tc.tile_pool(name="rpsum", bufs=2, space=bass.MemorySpace.PSUM))

        def load_tile(ft, t):
            r0 = _ROW_STARTS[t]
            nsp = 8 if t == T - 1 else (4 if t == T - 2 else DSPLIT)
            w = D // nsp
            for q in range(nsp):
                nc.gpsimd.dma_start(ft[:, q * w:(q + 1) * w],
                                    f_dram[r0:r0 + P, q * w:(q + 1) * w])

        ftiles = []
        for t in range(T):
            ft = fpool.tile([P, D], bf16, tag="f")
            if t < 3:
                load_tile(ft, t)
            ftiles.append(ft)

        identb = consts.tile([128, 128], bf16, tag="identb")
        masks.make_identity(nc, identb[:])
        m0_t = consts.tile([P, P], f32, tag="m0")
        pm_t = consts.tile([P, P], f32, tag="pm")
        im_t = consts.tile([P, P], f32, tag="im")
        w1_t = consts.tile([P, T], f32, tag="w1")
        w2_t = consts.tile([P, T], f32, tag="w2")
        nc.scalar.dma_start(m0_t[:], m0_dram[:, :])
        nc.scalar.dma_start(pm_t[:], pm_dram[:, :])
        nc.scalar.dma_start(im_t[:], im_dram[:, :])
        nc.scalar.dma_start(w1_t[:], w1_dram[:, :])
        nc.scalar.dma_start(w2_t[:], w2_dram[:, :])

        nb97 = consts.tile([P, 1], f32, tag="nb97")
        nc.vector.memset(nb97[:], -97.0)
        warm = consts.tile([1, 2], f32, tag="warm")
        nc.vector.memset(warm[:], 1.0)
        nc.scalar.activation(warm[:, 1:2], warm[:, 0:1], ACTF.Exp)

        t1cols = work.tile([P, T], f32, tag="t1cols")
        ldcols = work.tile([P, T], f32, tag="ldcols")
        egpool = ctx.enter_context(tc.tile_pool(name="egpool", bufs=4))
        egs = {}

        def tile_gram(t):
            ft = ftiles[t]
            if t >= 3:
                load_tile(ft, t)
            g = gpsum.tile([P, P], f32, tag="g")
            tcs_list = []
            # The last tile runs at 4-chunk granularity (aligned with its
            # 512-col load chunks) and interleaves MMs right behind each
            # group's copy, so only ~4 transposes + 4 MMs trail the final
            # DMA bytes.  Mid-run tiles keep 8-chunk groups and one long
            # MM burst.
            interleave = (t == T - 1)
            quad = 4 if interleave else QUAD
            for q in range(NCH // quad):
                tp = tpsum.tile([128, quad * P], bf16, tag="tp",
                                name=f"tp{t}_{q}")
                for j in range(quad):
                    c = q * quad + j
                    nc.tensor.transpose(
                        tp[:, j * P:(j + 1) * P],
                        ft[:, c * CH:(c + 1) * CH],
                        identb[:P, :P],
                    )
                tcs = tcpool.tile([128, quad * P], bf16, tag="tc",
                                  name=f"tc{t}_{q}")
                if q % 2 == 0:
                    nc.vector.tensor_copy(tcs[:], tp[:])
                else:
                    nc.scalar.copy(tcs[:], tp[:])
                tcs_list.append(tcs)
                if interleave:
                    for j in range(quad):
                        c = q * quad + j
                        sl = tcs[:, j * P:(j + 1) * P]
                        nc.tensor.matmul(g[:], sl, sl,
                                         start=(c == 0), stop=(c == NCH - 1))
            if not interleave:
                for c in range(NCH):
                    sl = tcs_list[c // quad][:, (c % quad) * P:(c % quad + 1) * P]
                    nc.tensor.matmul(g[:], sl, sl,
                                     start=(c == 0), stop=(c == NCH - 1))
            # d2 (the chain head) before the eg copy on the DVE FIFO.
            scr = lwork.tile([P, P], f32, tag="scr")
            nc.vector.tensor_tensor(scr[:], g[:], im_t[:], ALU.mult)
            d2 = small.tile([P, 1], f32, tag="d2")
            nc.vector.tensor_reduce(d2[:], scr[:], axis=AX.X, op=ALU.add)
            eg = egpool.tile([P, P], bf16, tag="eg")
            nc.vector.tensor_copy(eg[:], g[:])
            egs[t] = eg
            return d2

        def tile_softmax(t, d2):
            # rnx = (0.1*diag G)^-0.5; H = G @ diag(rnx) on the PE (G is
            # symmetric so lhsT=G).  hm = H*m0 zeroes self+cross entries,
            # so exp(rnx_i*hm_ij) is exp(logits) on kept entries and
            # exactly 1 on the 97 masked ones: the masked softmax
            # denominator is the plain accumulated row sum minus 97,
            # folded into the Ln bias.  The positives sum reads hm too
            # (pm keeps own-block off-diagonals only), with the row scale
            # applied after the reduce on a [P,1] vector.
            eg = egs.pop(t)
            lnv = small.tile([P, 1], f32, tag="lnv")
            nc.scalar.activation(lnv[:], d2[:], ACTF.Ln)
            rnx = small.tile([P, 1], f32, tag="rnx")
            nc.scalar.activation(rnx[:], lnv[:], ACTF.Exp, scale=-0.5)
            drn = lwork.tile([P, P], bf16, tag="drn")
            nc.vector.tensor_scalar(drn[:], im_t[:], rnx[:], 1.0 / TEMP,
                                    op0=ALU.mult, op1=ALU.mult)
            h_ps = rpsum.tile([P, P], f32, tag="r")
            nc.tensor.matmul(h_ps[:], eg[:], drn[:], start=True, stop=True)
            hm = lwork.tile([P, P], f32, tag="hm")
            nc.vector.tensor_tensor(hm[:], h_ps[:], m0_t[:], ALU.mult)
            xt = lwork.tile([P, P], f32, tag="xt")
            sr = small.tile([P, 1], f32, tag="sr")
            nc.scalar.activation(xt[:], hm[:], ACTF.Exp, scale=rnx[:],
                                 accum_out=sr[:])
            nc.scalar.activation(ldcols[:, t:t + 1], sr[:], ACTF.Ln,
                                 bias=nb97[:])
            lp = lwork.tile([P, P], f32, tag="lp")
            nc.vector.tensor_tensor(lp[:], hm[:], pm_t[:], ALU.mult)
            lr = small.tile([P, 1], f32, tag="lr")
            nc.vector.tensor_reduce(lr[:], lp[:], axis=AX.X, op=ALU.add)
            nc.vector.tensor_scalar_mul(t1cols[:, t:t + 1], lr[:], rnx[:])

        L = T - 1
        z1 = work.tile([P, T], f32, tag="z1")
        z2 = work.tile([P, T], f32, tag="z2")
        zs = work.tile([P, T], f32, tag="zs")
        zca = work.tile([P, 1], f32, tag="zca")
        for t in range(T):
            d2 = tile_gram(t)
            tile_softmax(t, d2)
            if t == T - 2:
                # Emit the bulk weighted reduce (cols 0..T-2) before the
                # last tile's chain so it runs under the stream tail -
                # engine queues are FIFO, late emission would serialize it
                # behind the final chain.
                nc.vector.tensor_tensor(z1[:, :L], t1cols[:, :L], w1_t[:, :L],
                                        ALU.mult)
                nc.vector.tensor_tensor(z2[:, :L], ldcols[:, :L], w2_t[:, :L],
                                        ALU.mult)
                nc.vector.tensor_add(zs[:, :L], z1[:, :L], z2[:, :L])
                nc.vector.tensor_reduce(zca[:], zs[:, :L], axis=AX.X,
                                        op=ALU.add)

        # Only the last tile's [120,1] contribution stays on the tail path.
        nc.vector.tensor_tensor(z1[:, L:], t1cols[:, L:], w1_t[:, L:], ALU.mult)
        nc.vector.tensor_tensor(z2[:, L:], ldcols[:, L:], w2_t[:, L:], ALU.mult)
        zsl = work.tile([P, 1], f32, tag="zsl")
        nc.vector.tensor_add(zsl[:], z1[:, L:], z2[:, L:])
        zc = work.tile([P, 1], f32, tag="zc")
        nc.vector.tensor_add(zc[:], zca[:], zsl[:])

        ones = work.tile([P, 1], f32, tag="ones")
        nc.vector.memset(ones[:], 1.0)
        tot_ps = gpsum.tile([1, 1], f32, tag="g")
        nc.tensor.matmul(tot_ps[:, :], zc[:], ones[:], start=True, stop=True)
        tot = work.tile([1, 1], f32, tag="tot")
        nc.vector.tensor_copy(tot[:], tot_ps[:, :])
        nc.sync.dma_start(out_dram[:, :], tot[:])

    nc.compile()
    return nc


def _ensure_axon_hooks():
    try:
        import antenv.axon_hooks  # noqa: F401
        return
    except ImportError:
        pass
    import contextlib
    import ctypes
    import types

    import antenv

    hook = None
    so_path = "/opt/axon/libaxon_pjrt.so"
    try:
        lib = ctypes.CDLL(so_path)
        if hasattr(lib, "axon_start_nrt_profile"):
            lib.axon_start_nrt_profile.argtypes = [
                ctypes.POINTER(ctypes.c_int64), ctypes.c_size_t]
            lib.axon_start_nrt_profile.restype = ctypes.c_int64
            lib.axon_stop_nrt_profile.argtypes = [ctypes.c_char_p]
            lib.axon_stop_nrt_profile.restype = ctypes.c_int64

            @contextlib.contextmanager
            def _hook(output_dir, device_ids):
                import jax
                jax.devices()
                if device_ids:
                    ids = (ctypes.c_int64 * len(device_ids))(*device_ids)
                    rc = lib.axon_start_nrt_profile(ids, len(device_ids))
                else:
                    rc = lib.axon_start_nrt_profile(None, 0)
                if rc != 0:
                    raise RuntimeError(f"axon_start_nrt_profile rc={rc}")
                try:
                    yield
                finally:
                    n = lib.axon_stop_nrt_profile(str(output_dir).encode())
                    print(f"profile: {n} file(s) written to {output_dir}",
                          file=sys.stderr)

            hook = _hook
    except OSError:
        pass

    mod = types.ModuleType("antenv.axon_hooks")
    state = {"hook": hook}
    mod.get_axon_ntff_profile_hook = lambda: state["hook"]
    mod.set_axon_ntff_profile_hook = lambda h: state.__setitem__("hook", h)
    sys.modules["antenv.axon_hooks"] = mod
    antenv.axon_hooks = mod


def kernel(**inputs):
    global _compiled, LAST_RESULTS
    from concourse import bass_utils

    feats = np.ascontiguousarray(
        np.asarray(inputs["features"], dtype=np.float32).reshape(B * M, D))

    if _compiled is None:
        _compiled = (_build(), _host_consts())
    nc, consts = _compiled

    in_maps = []
    for k in range(NCORES):
        im = dict(consts)
        im["f"] = feats[k * ROWS:(k + 1) * ROWS]
        in_maps.append(im)

    trace = bool(os.environ.get("BASS_TRACE"))
    if trace:
        _ensure_axon_hooks()
    try:
        res = bass_utils.run_bass_kernel_spmd(
            nc, in_maps, core_ids=list(range(NCORES)), trace=trace)
    except Exception:
        os.environ["BASS_NEVER_TRACE"] = "1"
        try:
            res = bass_utils.run_bass_kernel_spmd(
                nc, in_maps, core_ids=list(range(NCORES)), trace=False)
        finally:
            del os.environ["BASS_NEVER_TRACE"]
    LAST_RESULTS = res
    total = float(np.sum([np.float64(r["out"][0, 0]) for r in res.results]))
    return np.array(total, dtype=np.float32)
